# revision 1
# baseline (speedup 1.0000x reference)
"""Trainium2 Bass kernel for the DeepFermi deconvolution GD problem.

Reference computation (see problem statement): 10 fixed-step GD iterations on
a per-pixel objective

    F(eta) = ||ctc_dc - conv(aif_os, fermi_ir(eta))[::8]/8||^2 / C_dc
             + softplus(lambda) * ||(eta - eta_nn)||^2_Cnn + ||relu(-eta)||^2

The time-axis convolution with the (input-derived, iteration-independent) AIF
is a fixed 64x512 matrix M2; its transpose-products give all needed
reductions:

    s1    = sigmoid(k*(t0 - tsh))                 per pixel, [512]
    q     = M2 @ s1;   ctc_est = A*q
    r2    = (2/C_dc) * (A*q - ctc_dc)             [64]
    gA    = sum_j r2*q
    sd    = s1*(1-s1)
    U     = sum_j r2*(M2 @ sd);  V = sum_j r2*(M2V @ sd)   (M2V = M2*tsh)
    gk    = A*(t0*U - V);  gt0 = A*k*U

All pixels are independent; H(=128 rows) is sharded over the 8 cores, 16 rows
(2048 pixels) per core.  On-chip layout: pixels on partitions (one SBUF tile =
128 pixels x 512 time), sigmoid via one ScalarE activation with per-partition
scale/bias, PE transposes to feed the fixed-matrix matmuls, fused DVE
multiply-reduce ops for the dot products.
"""

import numpy as np

OSAMP = 8
MAX_ITER = 10
NEG_SHIFT = 2 * OSAMP
OTP = 5
C_SHARP = 500.0
LR = 0.1
T = 64
TOS = OSAMP * T  # 512
H = 128
W = 128
N_CORES = 8
ROWS_PER_CORE = H // N_CORES  # 16
TILES = ROWS_PER_CORE  # one 128-pixel tile per local H row
P = 128  # partitions


# ---------------------------------------------------------------------------
# host-side math (iteration independent; exact mirror of the reference's
# jax.image.resize 'linear' semantics)
# ---------------------------------------------------------------------------

def _resize_mat(in_size, out_size):
    """Column-stochastic linear-interp matrix [in, out] matching
    jax.image.resize(method='linear') for upsampling (antialias inactive)."""
    scale = out_size / in_size
    sample_f = (np.arange(out_size) + 0.5) / scale - 0.5
    x = np.abs(sample_f[None, :] - np.arange(in_size)[:, None])
    w = np.maximum(0.0, 1.0 - x)
    tot = w.sum(0, keepdims=True)
    w = np.where(np.abs(tot) > 1e-4, w / tot, 0.0)
    return w  # float64


def _sigmoid(x):
    return 1.0 / (1.0 + np.exp(-x))


def _preprocess(ctc, aif, time, eta_nn, lambda_reg):
    f64 = np.float64
    R = _resize_mat(T, TOS)
    aif0 = (aif.astype(f64) - aif.astype(f64)[..., :OTP].mean(-1, keepdims=True))
    ctc0 = (ctc.astype(f64) - ctc.astype(f64)[..., :OTP].mean(-1, keepdims=True))
    aif_os = (aif0 @ R)[0, 0, 0]                    # [512]
    t_os = time.astype(f64) @ R                     # [512]
    ctc_dc = (ctc0 @ R[:, ::OSAMP])[0]              # [H,W,64]
    C_dc = float((ctc_dc.astype(np.float32) ** 2).sum(dtype=np.float64))
    tsh = t_os - t_os[NEG_SHIFT]
    # fp32-faithful sharp step (saturates exactly like the fp32 reference)
    s2 = _sigmoid((C_SHARP * tsh).astype(np.float32).astype(f64))
    idx = NEG_SHIFT + 8 * np.arange(T)[:, None] - np.arange(TOS)[None, :]
    valid = (idx >= 0) & (idx <= TOS - 1)
    M = np.where(valid, aif_os[np.clip(idx, 0, TOS - 1)], 0.0) / OSAMP  # [64,512]
    M2 = M * s2[None, :]
    M2V = M2 * tsh[None, :]
    C_nn = (eta_nn.astype(f64) ** 2).sum(axis=(0, 2, 3))  # [3]
    sp_lam = np.logaddexp(0.0, float(lambda_reg.reshape(-1)[0]))
    creg = 2.0 * sp_lam / C_nn                      # [3]
    return M2, M2V, tsh, ctc_dc, C_dc, creg


# ---------------------------------------------------------------------------
# bass module (input-value independent; all data arrives via DRAM tensors)
# ---------------------------------------------------------------------------

_NC_CACHE = {}


def _build_nc():
    if "nc" in _NC_CACHE:
        return _NC_CACHE["nc"]

    import concourse.mybir as mybir
    import concourse.tile as tile
    from concourse import bacc

    dt = mybir.dt.float32
    bf = mybir.dt.bfloat16
    Alu = mybir.AluOpType
    Act = mybir.ActivationFunctionType

    nc = bacc.Bacc("TRN2", target_bir_lowering=False, debug=False)

    # shared constants (identical on every core)
    d_argw = nc.declare_dram_parameter("argw", [2 * TILES, 4 * TILES * P], bf,
                                       isOutput=False)
    d_ident = nc.declare_dram_parameter("ident", [P, P], bf, isOutput=False)
    d_m2t = nc.declare_dram_parameter("m2t", [P, 4 * T], bf, isOutput=False)
    d_muv = nc.declare_dram_parameter("muv", [P, 4 * 2 * T], bf, isOutput=False)
    # per-core data
    d_nctc = nc.declare_dram_parameter("negctc2", [P, TILES * T], dt, isOutput=False)
    d_eta0 = nc.declare_dram_parameter("eta0", [P, 3 * TILES], dt, isOutput=False)
    d_cpl48 = nc.declare_dram_parameter("cpl48", [P, 3 * TILES], dt, isOutput=False)
    d_s48 = nc.declare_dram_parameter("s48", [P, 3 * TILES], dt, isOutput=False)
    d_consts = nc.declare_dram_parameter("consts", [P, TILES], dt, isOutput=False)
    d_out = nc.declare_dram_parameter("out", [P, 3 * TILES], dt, isOutput=True)

    with tile.TileContext(nc) as tc:
        with (
            tc.tile_pool(name="const", bufs=1) as cpool,
            tc.tile_pool(name="state", bufs=2) as spool,
            tc.tile_pool(name="work", bufs=5) as wpool,
            tc.tile_pool(name="tpose", bufs=2) as tpool,
            tc.tile_pool(name="small", bufs=2) as mpool,
            tc.tile_pool(name="ps_t", bufs=3, space="PSUM") as ps_t,
            tc.tile_pool(name="ps_q", bufs=4, space="PSUM") as ps_q,
            tc.tile_pool(name="ps_k", bufs=1, space="PSUM") as ps_k,
        ):
            # ---- load constants ----
            argw = cpool.tile([2 * TILES, 4 * TILES * P], bf, tag="argw")
            nc.gpsimd.dma_start(argw[:], d_argw[:])
            ident = cpool.tile([P, P], bf, tag="ident")
            nc.gpsimd.dma_start(ident[:], d_ident[:])
            m2t = cpool.tile([P, 4 * T], bf, tag="m2t")
            nc.gpsimd.dma_start(m2t[:], d_m2t[:])
            muv = cpool.tile([P, 8 * T], bf, tag="muv")
            nc.gpsimd.dma_start(muv[:], d_muv[:])
            nctc = cpool.tile([P, TILES * T], dt, tag="nctc")
            nc.gpsimd.dma_start(nctc[:], d_nctc[:])
            cpl48 = cpool.tile([P, 3 * TILES], dt, tag="cpl48")
            nc.gpsimd.dma_start(cpl48[:], d_cpl48[:])
            s48 = cpool.tile([P, 3 * TILES], dt, tag="s48")
            nc.gpsimd.dma_start(s48[:], d_s48[:])
            consts = cpool.tile([P, TILES], dt, tag="consts")
            nc.gpsimd.dma_start(consts[:], d_consts[:])
            eta_in = cpool.tile([P, 3 * TILES], dt, tag="eta_in")
            nc.gpsimd.dma_start(eta_in[:], d_eta0[:])

            toc16 = consts[:, 0:TILES]

            # ---- initial eta state (A|k|t0 packed) + derived tiles ----
            eta48 = spool.tile([P, 3 * TILES], dt, tag="eta48")
            nc.vector.tensor_copy(eta48[:], eta_in[:])

            def make_derived(e48):
                eA = e48[:, 0:TILES]
                eK = e48[:, TILES:2 * TILES]
                eT = e48[:, 2 * TILES:3 * TILES]
                # kn[:, 2t] = (k*t0)_t, kn[:, 2t+1] = (-k)_t  (bf16), then
                # transpose so tile t's arg-matmul rhs is knT[2t:2t+2, :]
                kn = spool.tile([P, 2 * TILES], bf, tag="kn")
                nc.vector.tensor_tensor(kn[:, 0:2 * TILES:2], eK, eT,
                                        Alu.mult)
                nc.vector.tensor_scalar_mul(kn[:, 1:2 * TILES:2], eK, -1.0)
                knt_ps = ps_k.tile([2 * TILES, P], bf, tag="kntp")
                nc.tensor.transpose(knt_ps[:], kn[:], ident[:])
                knT = spool.tile([2 * TILES, P], bf, tag="knT")
                nc.scalar.copy(knT[:], knt_ps[:])
                a2c = spool.tile([P, TILES], dt, tag="a2c")
                nc.vector.tensor_tensor(a2c[:], eA[:], toc16, Alu.mult)
                return knT, a2c

            knT, a2c = make_derived(eta48)

            for it in range(MAX_ITER):
                G48 = mpool.tile([P, 3 * TILES], dt, tag="G48")
                accGA = G48[:, 0:TILES]
                accU = mpool.tile([P, TILES], dt, tag="accU")
                accV = mpool.tile([P, TILES], dt, tag="accV")

                for t in range(TILES):
                    # argT[v,p] = kt0_p - k_p*tsh_v via rank-2 matmul:
                    # lhsT = [ones; tsh] chunk, rhs = knT[2t:2t+2, :]
                    argp = ps_t.tile([P, TOS], dt, tag="argp")
                    for c in range(4):
                        blk = 4 * t + c
                        nc.tensor.matmul(
                            argp[:, c * P:(c + 1) * P],
                            argw[:, blk * P:(blk + 1) * P],
                            knT[:],
                            start=True, stop=True,
                        )
                    # s1T = sigmoid(argT)  (PSUM -> SBUF, bf16 out)
                    s1T = wpool.tile([P, TOS], bf, tag="s1T")
                    nc.scalar.activation(s1T[:], argp[:], Act.Sigmoid)
                    # sdT = s1T*(1-s1T)
                    sdT = wpool.tile([P, TOS], bf, tag="sdT")
                    sdacc = wpool.tile([P, 1], dt, tag="sdacc")
                    nc.vector.affine_mul_reduce(
                        sdT[:], sdacc[:], s1T[:], s1T[:], -1.0, 1.0,
                    )

                    # q = M2 @ s1 -> [128p, 64]; qd|qdv = (M2|M2V) @ sd -> [128p, 128]
                    qq = ps_q.tile([P, 3 * T], dt, tag="qq")
                    for c in range(4):
                        nc.tensor.matmul(
                            qq[:, 0:T], s1T[:, c * P:(c + 1) * P],
                            m2t[:, c * T:(c + 1) * T],
                            start=(c == 0), stop=(c == 3),
                        )
                    for c in range(4):
                        nc.tensor.matmul(
                            qq[:, T: 3 * T], sdT[:, c * P:(c + 1) * P],
                            muv[:, c * 2 * T:(c + 1) * 2 * T],
                            start=(c == 0), stop=(c == 3),
                        )
                    # single PSUM->SBUF copy (bf16) for all of q|qd|qdv
                    qqs = wpool.tile([P, 3 * T], bf, tag="qqs")
                    nc.scalar.copy(qqs[:], qq[:])
                    q_ap = qqs[:, 0:T]
                    qd_ap = qqs[:, T: 2 * T]
                    qdv_ap = qqs[:, 2 * T: 3 * T]

                    # r2 = (2A/C_dc)*q - (2/C_dc)*ctc_dc
                    r2 = wpool.tile([P, T], dt, tag="r2")
                    nc.vector.affine_then_add(
                        r2[:], q_ap, nctc[:, t * T:(t + 1) * T],
                        a2c[:, t:t + 1], 0.0,
                    )
                    # dots: accGA[:,t] = sum r2*q ; accU ; accV   (seed 0;
                    # the cpa prior-term is added during the combine phase)
                    dsc = wpool.tile([P, 3 * T], dt, tag="dsc")
                    nc.vector.affine_mul_reduce(
                        dsc[:, 0:T], accGA[:, t:t + 1], q_ap, r2[:], 1.0, 0.0)
                    nc.vector.affine_mul_reduce(
                        dsc[:, T: 2 * T], accU[:, t:t + 1], qd_ap, r2[:], 1.0, 0.0)
                    nc.vector.affine_mul_reduce(
                        dsc[:, 2 * T: 3 * T], accV[:, t:t + 1], qdv_ap, r2[:], 1.0, 0.0)

                # ---- combine: eta <- eta - LR*grad, batched [128,48] ----
                # products chain (GpSimd, idle engine): G48 cols 16:48
                eA = eta48[:, 0:TILES]
                eK = eta48[:, TILES:2 * TILES]
                eT = eta48[:, 2 * TILES:3 * TILES]
                p1 = mpool.tile([P, TILES], dt, tag="p1")
                nc.gpsimd.tensor_tensor(p1[:], eA, accU[:], Alu.mult)
                p2 = mpool.tile([P, TILES], dt, tag="p2")
                nc.gpsimd.tensor_tensor(p2[:], eA, accV[:], Alu.mult)
                wk = mpool.tile([P, TILES], dt, tag="wk")
                nc.gpsimd.tensor_tensor(wk[:], eT, p1[:], Alu.mult)
                nc.gpsimd.tensor_tensor(G48[:, TILES:2 * TILES], wk[:], p2[:],
                                        Alu.subtract)
                nc.gpsimd.tensor_tensor(G48[:, 2 * TILES:3 * TILES], p1[:], eK,
                                        Alu.mult)
                # DVE: m48 = -2LR*min(eta,0); eta' = eta*s48 - LR*G48 + m48 + cpl48
                m48 = mpool.tile([P, 3 * TILES], dt, tag="m48")
                nc.vector.tensor_scalar(m48[:], eta48[:], 0.0, -2.0 * LR,
                                        Alu.min, Alu.mult)
                t48 = mpool.tile([P, 3 * TILES], dt, tag="t48")
                nc.vector.affine_then_add(t48[:], G48[:], m48[:], -LR, 0.0)
                t48b = mpool.tile([P, 3 * TILES], dt, tag="t48b")
                nc.vector.tensor_tensor(t48b[:], t48[:], cpl48[:], Alu.add)
                up48 = mpool.tile([P, 3 * TILES], dt, tag="up48")
                nc.vector.tensor_tensor(up48[:], eta48[:], s48[:], Alu.mult)
                eta48n = spool.tile([P, 3 * TILES], dt, tag="eta48")
                nc.vector.tensor_tensor(eta48n[:], up48[:], t48b[:], Alu.add)

                eta48 = eta48n
                if it < MAX_ITER - 1:
                    knT, a2c = make_derived(eta48)

            nc.gpsimd.dma_start(d_out[:], eta48[:])

    nc.finalize()
    _NC_CACHE["nc"] = nc
    return nc


# ---------------------------------------------------------------------------
# public entry point
# ---------------------------------------------------------------------------

def _make_in_maps(ctc, aif, time, eta_nn, lambda_reg):
    f32 = np.float32
    M2, M2V, tsh, ctc_dc, C_dc, creg = _preprocess(ctc, aif, time, eta_nn, lambda_reg)

    toc = 2.0 / C_dc
    sA, sK, sT0 = (1.0 - LR * creg).astype(np.float64)

    import ml_dtypes
    bf16 = ml_dtypes.bfloat16
    # argw[r, 128*(4t+c)+vv] = 1 if r==2t else tsh[128c+vv] if r==2t+1 else 0
    argw = np.zeros((2 * TILES, 4 * TILES * P), bf16)
    tshf = tsh.astype(np.float32)
    for t_ in range(TILES):
        for c_ in range(4):
            blk = 4 * t_ + c_
            argw[2 * t_, blk * P:(blk + 1) * P] = 1.0
            argw[2 * t_ + 1, blk * P:(blk + 1) * P] = tshf[c_ * P:(c_ + 1) * P]
    ident = np.eye(P, dtype=bf16)
    # m2t[vv, 64c+j] = M2[j, 128c+vv];  muv[vv, 128c+j'] = (M2|M2V)[j', 128c+vv]
    m2t = np.zeros((P, 4 * T), bf16)
    muv = np.zeros((P, 8 * T), bf16)
    for c in range(4):
        blk = M2[:, c * P:(c + 1) * P]       # [64,128]
        blkv = M2V[:, c * P:(c + 1) * P]
        m2t[:, c * T:(c + 1) * T] = blk.T
        muv[:, c * 2 * T: c * 2 * T + T] = blk.T
        muv[:, c * 2 * T + T: (c + 1) * 2 * T] = blkv.T

    consts = np.full((P, TILES), toc, f32)
    s48 = np.zeros((P, 3 * TILES), f32)
    s48[:, 0:TILES] = sA
    s48[:, TILES:2 * TILES] = sK
    s48[:, 2 * TILES:] = sT0

    in_maps = []
    for m in range(N_CORES):
        rows = slice(m * ROWS_PER_CORE, (m + 1) * ROWS_PER_CORE)
        # ctc_dc[h, w, j]: tile t = local row, partition p = w
        cd = ctc_dc[rows]                     # [16, 128, 64]
        negctc2 = np.ascontiguousarray(
            (-toc * cd).transpose(1, 0, 2).reshape(P, TILES * T)).astype(f32)
        pr = eta_nn[0, :, rows, :].astype(np.float64)   # [3, 16, 128] (c, t, p)
        eta0 = np.ascontiguousarray(
            pr.transpose(2, 0, 1).reshape(P, 3 * TILES)).astype(f32)
        cpl48 = np.zeros((P, 3 * TILES), f32)
        for c in range(3):
            cpl48[:, c * TILES:(c + 1) * TILES] = (LR * creg[c] * pr[c]).T
        in_maps.append({
            "argw": argw, "ident": ident, "m2t": m2t, "muv": muv,
            "negctc2": negctc2, "eta0": eta0, "cpl48": cpl48, "s48": s48,
            "consts": consts,
        })
    return in_maps


def kernel(ctc, aif, time, seg, eta_nn, lambda_reg):
    from concourse.bass_utils import run_bass_kernel_spmd

    ctc = np.asarray(ctc)
    aif = np.asarray(aif)
    time = np.asarray(time)
    eta_nn = np.asarray(eta_nn)
    lambda_reg = np.asarray(lambda_reg)

    in_maps = _make_in_maps(ctc, aif, time, eta_nn, lambda_reg)
    nc = _build_nc()
    res = run_bass_kernel_spmd(nc, in_maps, list(range(N_CORES)))

    out = np.zeros((1, 3, H, W), np.float32)
    for m in range(N_CORES):
        rows = slice(m * ROWS_PER_CORE, (m + 1) * ROWS_PER_CORE)
        arr = res.results[m]["out"]                  # [128, 48]
        out[0, :, rows, :] = arr.reshape(P, 3, TILES).transpose(1, 2, 0)
    return out



# revision 12
# speedup vs baseline: 2.6188x; 2.6188x over previous
"""Trainium2 Bass kernel for the DeepFermi deconvolution GD problem (v3).

Reference: 10 fixed-step GD iterations of a per-pixel objective; per pixel
(A, k, t0) with s1 = sigmoid(k*(t0 - tsh)) on a 512-point oversampled grid,
q = M2 @ s1, r2 = (2/C)(A q - ctc_dc), and gradient dots r2.q, r2.(M2 sd),
r2.(M2V sd).

Reformulation (validated numerically, rel err ~2e-4 << 2e-2 budget):
  1. Coarsen the time grid 512 -> nB=16 blocks (M2 block-summed exactly; the
     sharp C=500 step is inside M2; sigma evaluated at block centers cB).
  2. Bilinear Gram form eliminates q/r2 entirely:
        D1 = s1.Qq.s1, D2 = s1.Qq.sd, D3 = s1.Qv.sd   (Qq=toc*M2b'M2b etc.)
        D4 = wq.s1, D5 = wq.sd, D6 = wv.sd             (wq=toc*M2b'cd fixed)
        gA = A*D1-D4; U = A*D2-D5; V = A*D3-D6; gk = A(t0*U-V); gt0 = A*k*U
  3. Four tiles (H-rows) per 128-partition quad (32-row slots); one K=32
     rank-2 matmul builds arg (weight col picks the tile's knT rows), one
     K=128 matmul with a block-diagonal Gram stack gives compact
     y = [yq|yq|yv] (48 valid cols/tile), one PE transpose gives sigma
     pixel-major.  Dots are two batched DVE mult + reduce pairs per
     half-iteration over an [s1|sd|sd] triple layout.
  4. Everything depending only on eta (A3, A*k, eta*s48+cpl, -2LR*min(eta,0))
     is hoisted to iteration start, off the serial tail.
"""

import numpy as np

OSAMP = 8
MAX_ITER = 10
NEG_SHIFT = 2 * OSAMP
OTP = 5
C_SHARP = 500.0
LR = 0.1
T = 64
TOS = OSAMP * T      # 512
H = 128
W = 128
N_CORES = 8
ROWS_PER_CORE = H // N_CORES  # 16
TILES = ROWS_PER_CORE
P = 128
BLK = 32             # partition slot width per tile
NB = 16              # time blocks
NQ = 4               # quads (4 tiles each)


# ---------------------------------------------------------------------------
# host-side math (iteration independent)
# ---------------------------------------------------------------------------

def _resize_mat(in_size, out_size):
    scale = out_size / in_size
    sample_f = (np.arange(out_size) + 0.5) / scale - 0.5
    x = np.abs(sample_f[None, :] - np.arange(in_size)[:, None])
    w = np.maximum(0.0, 1.0 - x)
    tot = w.sum(0, keepdims=True)
    w = np.where(np.abs(tot) > 1e-4, w / tot, 0.0)
    return w


def _sigmoid(x):
    return 1.0 / (1.0 + np.exp(-np.clip(x, -80, 80)))


def _preprocess(ctc, aif, time, eta_nn, lambda_reg):
    f64 = np.float64
    R = _resize_mat(T, TOS)
    aif0 = (aif.astype(f64) - aif.astype(f64)[..., :OTP].mean(-1, keepdims=True))
    ctc0 = (ctc.astype(f64) - ctc.astype(f64)[..., :OTP].mean(-1, keepdims=True))
    aif_os = (aif0 @ R)[0, 0, 0]
    t_os = time.astype(f64) @ R
    ctc_dc = (ctc0 @ R[:, ::OSAMP])[0]              # [H,W,64]
    C_dc = float((ctc_dc.astype(np.float32) ** 2).sum(dtype=np.float64))
    tsh = t_os - t_os[NEG_SHIFT]
    s2 = _sigmoid((C_SHARP * tsh).astype(np.float32).astype(f64))
    idx = NEG_SHIFT + 8 * np.arange(T)[:, None] - np.arange(TOS)[None, :]
    valid = (idx >= 0) & (idx <= TOS - 1)
    M = np.where(valid, aif_os[np.clip(idx, 0, TOS - 1)], 0.0) / OSAMP
    M2 = M * s2[None, :]
    M2V = M2 * tsh[None, :]
    C_nn = (eta_nn.astype(f64) ** 2).sum(axis=(0, 2, 3))  # [3]
    sp_lam = np.logaddexp(0.0, float(lambda_reg.reshape(-1)[0]))
    creg = 2.0 * sp_lam / C_nn
    return M2, M2V, tsh, ctc_dc, C_dc, creg


# ---------------------------------------------------------------------------
# bass module
# ---------------------------------------------------------------------------

_NC_CACHE = {}


def _build_nc():
    if "nc" in _NC_CACHE:
        return _NC_CACHE["nc"]

    import concourse.mybir as mybir
    import concourse.tile as tile
    from concourse import bacc

    dt = mybir.dt.float32
    bf = mybir.dt.bfloat16
    Alu = mybir.AluOpType
    Act = mybir.ActivationFunctionType
    Ax = mybir.AxisListType

    nc = bacc.Bacc("TRN2", target_bir_lowering=False, debug=False)

    # shared constants
    d_argw = nc.declare_dram_parameter("argw", [2 * TILES, NQ * P], bf,
                                       isOutput=False)
    d_ident = nc.declare_dram_parameter("ident", [P, P], bf, isOutput=False)
    d_qqv3 = nc.declare_dram_parameter("qqv3", [P, 4 * 3 * NB], bf,
                                       isOutput=False)
    # per-core data
    d_w3h = nc.declare_dram_parameter("w3h", [P, TILES * 3 * NB], bf,
                                      isOutput=False)
    d_eta0 = nc.declare_dram_parameter("eta0", [P, 3 * TILES], dt,
                                       isOutput=False)
    d_cpl48 = nc.declare_dram_parameter("cpl48", [P, 3 * TILES], dt,
                                        isOutput=False)
    d_s48 = nc.declare_dram_parameter("s48", [P, 3 * TILES], dt, isOutput=False)
    d_out = nc.declare_dram_parameter("out", [P, 3 * TILES], dt, isOutput=True)

    with tile.TileContext(nc) as tc:
        with (
            tc.tile_pool(name="const", bufs=1) as cpool,
            tc.tile_pool(name="state", bufs=2) as spool,
            tc.tile_pool(name="iter", bufs=2) as ipool,
            tc.tile_pool(name="quad", bufs=4) as qpool,
            tc.tile_pool(name="small", bufs=2) as mpool,
            tc.tile_pool(name="ps_arg", bufs=2, space="PSUM") as ps_arg,
            tc.tile_pool(name="ps_y3", bufs=3, space="PSUM") as ps_y3,
            tc.tile_pool(name="ps_spx", bufs=2, space="PSUM") as ps_spx,
            tc.tile_pool(name="ps_kn", bufs=1, space="PSUM") as ps_kn,
        ):
            # ---- load constants ----
            argw = cpool.tile([2 * TILES, NQ * P], bf, tag="argw")
            nc.gpsimd.dma_start(argw[:], d_argw[:])
            ident = cpool.tile([P, P], bf, tag="ident")
            nc.gpsimd.dma_start(ident[:], d_ident[:])
            qqv3 = cpool.tile([P, 4 * 3 * NB], bf, tag="qqv3")
            nc.gpsimd.dma_start(qqv3[:], d_qqv3[:])
            w3h = cpool.tile([P, TILES * 3 * NB], bf, tag="w3h")
            nc.gpsimd.dma_start(w3h[:], d_w3h[:])
            cpl48 = cpool.tile([P, 3 * TILES], dt, tag="cpl48")
            nc.gpsimd.dma_start(cpl48[:], d_cpl48[:])
            s48 = cpool.tile([P, 3 * TILES], dt, tag="s48")
            nc.gpsimd.dma_start(s48[:], d_s48[:])
            eta_in = cpool.tile([P, 3 * TILES], dt, tag="eta_in")
            nc.gpsimd.dma_start(eta_in[:], d_eta0[:])

            eta48 = spool.tile([P, 3 * TILES], dt, tag="eta48")
            nc.vector.tensor_copy(eta48[:], eta_in[:])

            for it in range(MAX_ITER):
                eA = eta48[:, 0:TILES]
                eK = eta48[:, TILES:2 * TILES]
                eT = eta48[:, 2 * TILES:3 * TILES]

                # --- eta-only prefolds (off the serial tail) ---
                # kn[:, 2t]=(k*t0)_t, kn[:, 2t+1]=(-k)_t; knT = kn^T
                kn = ipool.tile([P, 2 * TILES], bf, tag="kn")
                nc.vector.tensor_tensor(kn[:, 0:2 * TILES:2], eK, eT, Alu.mult)
                nc.vector.tensor_scalar_mul(kn[:, 1:2 * TILES:2], eK, -1.0)
                knt_ps = ps_kn.tile([2 * TILES, P], bf, tag="kntp")
                nc.tensor.transpose(knt_ps[:], kn[:], ident[:])
                knT = ipool.tile([2 * TILES, P], bf, tag="knT")
                nc.scalar.copy(knT[:], knt_ps[:])
                # A3 = [A|A|A], AK = A*k  (GpSimd, parallel)
                A3 = ipool.tile([P, 3 * TILES], dt, tag="A3")
                for c in range(3):
                    nc.gpsimd.tensor_copy(A3[:, c * TILES:(c + 1) * TILES], eA)
                AK = ipool.tile([P, TILES], dt, tag="AK")
                nc.gpsimd.tensor_tensor(AK[:], eA, eK, Alu.mult)
                # upc = eta*s48 + cpl48 ; m48 = -2LR*min(eta,0)
                ups = ipool.tile([P, 3 * TILES], dt, tag="ups")
                nc.gpsimd.tensor_tensor(ups[:], eta48[:], s48[:], Alu.mult)
                upc = ipool.tile([P, 3 * TILES], dt, tag="upc")
                nc.gpsimd.tensor_tensor(upc[:], ups[:], cpl48[:], Alu.add)
                m48 = ipool.tile([P, 3 * TILES], dt, tag="m48")
                nc.gpsimd.tensor_scalar(m48[:], eta48[:], 0.0, -2.0 * LR,
                                        Alu.min, Alu.mult)

                # --- per-quad pipeline: arg -> sigma -> {y, sigma^T} ---
                ssd3 = ipool.tile([P, TILES * 3 * NB], bf, tag="ssd3")
                y3s = ipool.tile([P, TILES * 3 * NB], bf, tag="y3s")
                sdacc = ipool.tile([P, NQ], dt, tag="sdacc")
                for q in range(NQ):
                    argp = ps_arg.tile([P, P], dt, tag="argp")
                    nc.tensor.matmul(
                        argp[:], argw[:, q * P:(q + 1) * P], knT[:],
                        start=True, stop=True,
                    )
                    s1Tq = qpool.tile([P, P], bf, tag="s1Tq")
                    nc.scalar.activation(s1Tq[:], argp[:], Act.Sigmoid)
                    y3p = ps_y3.tile([P, 4 * 3 * NB], dt, tag="y3p")
                    nc.tensor.matmul(
                        y3p[:], s1Tq[:], qqv3[:],
                        start=True, stop=True,
                    )
                    spxp = ps_spx.tile([P, P], bf, tag="spxp")
                    nc.tensor.transpose(spxp[:], s1Tq[:], ident[:])

                    qof = q * 4 * 3 * NB
                    s3q = ssd3[:, qof:qof + 4 * 3 * NB] \
                        .rearrange("p (a c b) -> p a c b", a=4, c=3)
                    # s1 (valid 16 of each 32-slot)
                    nc.scalar.copy(
                        s3q[:, :, 0, :],
                        spxp[:].rearrange("p (a b) -> p a b", a=4)[:, :, 0:NB],
                    )
                    # y3s: compact copy (all 192 cols valid)
                    nc.scalar.copy(y3s[:, qof:qof + 4 * 3 * NB], y3p[:])
                    # sd = s1*(1-s1)
                    nc.vector.affine_mul_reduce(
                        s3q[:, :, 1, :], sdacc[:, q:q + 1],
                        s3q[:, :, 0, :], s3q[:, :, 0, :], -1.0, 1.0,
                    )
                    # duplicate sd into slot 2 (GpSimd, off DVE)
                    nc.gpsimd.tensor_copy(s3q[:, :, 2, :], s3q[:, :, 1, :])

                # --- dots per half (8 tiles): two mult+reduce pairs ---
                # DD padded to 112 so the strided out-view slices stay
                # in-bounds; layout DD[:, d*16 + t], d in 0..5
                DD = mpool.tile([P, 7 * TILES], dt, tag="DD")
                for h in range(2):
                    hof = h * 8 * 3 * NB
                    hsl = slice(hof, hof + 8 * 3 * NB)
                    ph1 = mpool.tile([P, 8 * 3 * NB], bf, tag=f"ph1_{h}")
                    nc.vector.tensor_tensor(ph1[:], y3s[:, hsl], ssd3[:, hsl],
                                            Alu.mult)
                    ph2 = mpool.tile([P, 8 * 3 * NB], bf, tag=f"ph2_{h}")
                    nc.vector.tensor_tensor(ph2[:], w3h[:, hsl], ssd3[:, hsl],
                                            Alu.mult)
                    # DD[:, d*16 + (8h+t)] <- sum_B ph[:, t, d, B]
                    nc.vector.tensor_reduce(
                        DD[:, 8 * h:8 * h + 48]
                        .rearrange("p (c t) -> p t c", c=3)[:, 0:8, :],
                        ph1[:].rearrange("p (t c b) -> p t c b", t=8, c=3),
                        Ax.X, Alu.add)
                    nc.vector.tensor_reduce(
                        DD[:, 3 * TILES + 8 * h:3 * TILES + 8 * h + 48]
                        .rearrange("p (c t) -> p t c", c=3)[:, 0:8, :],
                        ph2[:].rearrange("p (t c b) -> p t c b", t=8, c=3),
                        Ax.X, Alu.add)

                # --- combine on DVE (short chain) ---
                GUp = mpool.tile([P, 3 * TILES], dt, tag="GUp")
                nc.vector.tensor_tensor(GUp[:], A3[:], DD[:, 0:3 * TILES],
                                        Alu.mult)
                GU = mpool.tile([P, 3 * TILES], dt, tag="GU")
                nc.vector.tensor_tensor(GU[:], GUp[:],
                                        DD[:, 3 * TILES:6 * TILES],
                                        Alu.subtract)
                U_ap = GU[:, TILES:2 * TILES]
                V_ap = GU[:, 2 * TILES:3 * TILES]
                G48 = mpool.tile([P, 3 * TILES], dt, tag="G48")
                nc.vector.tensor_copy(G48[:, 0:TILES], GU[:, 0:TILES])
                nc.vector.tensor_tensor(G48[:, 2 * TILES:3 * TILES], AK[:],
                                        U_ap, Alu.mult)
                m1 = mpool.tile([P, TILES], dt, tag="m1")
                nc.vector.tensor_tensor(m1[:], eT, U_ap, Alu.mult)
                m2 = mpool.tile([P, TILES], dt, tag="m2")
                nc.vector.tensor_tensor(m2[:], m1[:], V_ap, Alu.subtract)
                nc.vector.tensor_tensor(G48[:, TILES:2 * TILES], eA, m2[:],
                                        Alu.mult)

                # --- update: eta' = upc + (G48*(-LR) + m48) ---
                t48 = mpool.tile([P, 3 * TILES], dt, tag="t48")
                nc.vector.affine_then_add(t48[:], G48[:], m48[:], -LR, 0.0)
                eta48n = spool.tile([P, 3 * TILES], dt, tag="eta48")
                nc.vector.tensor_tensor(eta48n[:], upc[:], t48[:], Alu.add)
                eta48 = eta48n

            nc.gpsimd.dma_start(d_out[:], eta48[:])

    nc.finalize()
    _NC_CACHE["nc"] = nc
    return nc


# ---------------------------------------------------------------------------
# public entry point
# ---------------------------------------------------------------------------

def _make_in_maps(ctc, aif, time, eta_nn, lambda_reg):
    f32 = np.float32
    M2, M2V, tsh, ctc_dc, C_dc, creg = _preprocess(
        ctc, aif, time, eta_nn, lambda_reg)

    toc = 2.0 / C_dc
    M2b = M2.reshape(T, NB, TOS // NB).sum(-1)     # [64, 16]
    M2Vb = M2V.reshape(T, NB, TOS // NB).sum(-1)
    cB = tsh.reshape(NB, TOS // NB).mean(-1)       # [16]
    Qq = toc * (M2b.T @ M2b)                       # [16, 16]
    Qv = toc * (M2b.T @ M2Vb)

    import ml_dtypes
    bf16 = ml_dtypes.bfloat16

    # argw[r, q*128 + 32a + B]: tile t = 4q+a; r==2t -> 1, r==2t+1 -> cB[B]
    cBp = cB[np.minimum(np.arange(BLK), NB - 1)]   # padded to 32
    argw = np.zeros((2 * TILES, NQ * P), bf16)
    for q_ in range(NQ):
        for a_ in range(4):
            t_ = 4 * q_ + a_
            col = q_ * P + 32 * a_
            argw[2 * t_, col:col + 32] = 1.0
            argw[2 * t_ + 1, col:col + 32] = cBp.astype(bf16)
    ident = np.eye(P, dtype=bf16)
    # block-diagonal compact Gram stack:
    # qqv3[32a+B, 48a + 16d + j] = [Qq|Qq|Qv][B, 16d+j], B < 16
    blkrow = np.concatenate([Qq, Qq, Qv], axis=1)          # [16, 48]
    qqv3 = np.zeros((P, 4 * 3 * NB), bf16)                 # [128, 192]
    for a_ in range(4):
        qqv3[32 * a_:32 * a_ + NB, 48 * a_:48 * a_ + 48] = \
            blkrow.astype(bf16)

    s48 = np.zeros((P, 3 * TILES), f32)
    for c in range(3):
        s48[:, c * TILES:(c + 1) * TILES] = 1.0 - LR * creg[c]

    in_maps = []
    for m in range(N_CORES):
        rows = slice(m * ROWS_PER_CORE, (m + 1) * ROWS_PER_CORE)
        cd = ctc_dc[rows]                         # [16, 128, 64]
        wq = toc * (cd @ M2b)                     # [16, 128, 16]
        wv = toc * (cd @ M2Vb)
        wq_pm = wq.transpose(1, 0, 2)             # [128, 16, 16]
        wv_pm = wv.transpose(1, 0, 2)
        w3h = np.ascontiguousarray(
            np.stack([wq_pm, wq_pm, wv_pm], axis=2)
            .reshape(P, TILES * 3 * NB)).astype(bf16)
        pr = eta_nn[0, :, rows, :].astype(np.float64)   # [3, 16, 128]
        eta0 = np.ascontiguousarray(
            pr.transpose(2, 0, 1).reshape(P, 3 * TILES)).astype(f32)
        cpl48 = np.zeros((P, 3 * TILES), f32)
        for c in range(3):
            cpl48[:, c * TILES:(c + 1) * TILES] = (LR * creg[c] * pr[c]).T
        in_maps.append({
            "argw": argw, "ident": ident, "qqv3": qqv3, "w3h": w3h,
            "eta0": eta0, "cpl48": cpl48, "s48": s48,
        })
    return in_maps


def kernel(ctc, aif, time, seg, eta_nn, lambda_reg):
    from concourse.bass_utils import run_bass_kernel_spmd

    ctc = np.asarray(ctc)
    aif = np.asarray(aif)
    time = np.asarray(time)
    eta_nn = np.asarray(eta_nn)
    lambda_reg = np.asarray(lambda_reg)

    in_maps = _make_in_maps(ctc, aif, time, eta_nn, lambda_reg)
    nc = _build_nc()
    res = run_bass_kernel_spmd(nc, in_maps, list(range(N_CORES)))

    out = np.zeros((1, 3, H, W), np.float32)
    for m in range(N_CORES):
        rows = slice(m * ROWS_PER_CORE, (m + 1) * ROWS_PER_CORE)
        arr = res.results[m]["out"]                  # [128, 48]
        out[0, :, rows, :] = arr.reshape(P, 3, TILES).transpose(1, 2, 0)
    return out


# revision 15
# speedup vs baseline: 2.7006x; 1.0313x over previous
"""Trainium2 Bass kernel for the DeepFermi deconvolution GD problem (v3).

Reference: 10 fixed-step GD iterations of a per-pixel objective; per pixel
(A, k, t0) with s1 = sigmoid(k*(t0 - tsh)) on a 512-point oversampled grid,
q = M2 @ s1, r2 = (2/C)(A q - ctc_dc), and gradient dots r2.q, r2.(M2 sd),
r2.(M2V sd).

Reformulation (validated numerically, rel err ~2e-4 << 2e-2 budget):
  1. Coarsen the time grid 512 -> nB=16 blocks (M2 block-summed exactly; the
     sharp C=500 step is inside M2; sigma evaluated at block centers cB).
  2. Bilinear Gram form eliminates q/r2 entirely:
        D1 = s1.Qq.s1, D2 = s1.Qq.sd, D3 = s1.Qv.sd   (Qq=toc*M2b'M2b etc.)
        D4 = wq.s1, D5 = wq.sd, D6 = wv.sd             (wq=toc*M2b'cd fixed)
        gA = A*D1-D4; U = A*D2-D5; V = A*D3-D6; gk = A(t0*U-V); gt0 = A*k*U
  3. Four tiles (H-rows) per 128-partition quad (32-row slots); one K=32
     rank-2 matmul builds arg (weight col picks the tile's knT rows), one
     K=128 matmul with a block-diagonal Gram stack gives compact
     y = [yq|yq|yv] (48 valid cols/tile), one PE transpose gives sigma
     pixel-major.  Dots are two batched DVE mult + reduce pairs per
     half-iteration over an [s1|sd|sd] triple layout.
  4. Everything depending only on eta (A3, A*k, eta*s48+cpl, -2LR*min(eta,0))
     is hoisted to iteration start, off the serial tail.
"""

import numpy as np

OSAMP = 8
MAX_ITER = 10
NEG_SHIFT = 2 * OSAMP
OTP = 5
C_SHARP = 500.0
LR = 0.1
T = 64
TOS = OSAMP * T      # 512
H = 128
W = 128
N_CORES = 8
ROWS_PER_CORE = H // N_CORES  # 16
TILES = ROWS_PER_CORE
P = 128
BLK = 32             # partition slot width per tile
NB = 16              # time blocks
NQ = 4               # quads (4 tiles each)


# ---------------------------------------------------------------------------
# host-side math (iteration independent)
# ---------------------------------------------------------------------------

def _resize_mat(in_size, out_size):
    scale = out_size / in_size
    sample_f = (np.arange(out_size) + 0.5) / scale - 0.5
    x = np.abs(sample_f[None, :] - np.arange(in_size)[:, None])
    w = np.maximum(0.0, 1.0 - x)
    tot = w.sum(0, keepdims=True)
    w = np.where(np.abs(tot) > 1e-4, w / tot, 0.0)
    return w


def _sigmoid(x):
    return 1.0 / (1.0 + np.exp(-np.clip(x, -80, 80)))


def _preprocess(ctc, aif, time, eta_nn, lambda_reg):
    f64 = np.float64
    R = _resize_mat(T, TOS)
    aif0 = (aif.astype(f64) - aif.astype(f64)[..., :OTP].mean(-1, keepdims=True))
    ctc0 = (ctc.astype(f64) - ctc.astype(f64)[..., :OTP].mean(-1, keepdims=True))
    aif_os = (aif0 @ R)[0, 0, 0]
    t_os = time.astype(f64) @ R
    ctc_dc = (ctc0 @ R[:, ::OSAMP])[0]              # [H,W,64]
    C_dc = float((ctc_dc.astype(np.float32) ** 2).sum(dtype=np.float64))
    tsh = t_os - t_os[NEG_SHIFT]
    s2 = _sigmoid((C_SHARP * tsh).astype(np.float32).astype(f64))
    idx = NEG_SHIFT + 8 * np.arange(T)[:, None] - np.arange(TOS)[None, :]
    valid = (idx >= 0) & (idx <= TOS - 1)
    M = np.where(valid, aif_os[np.clip(idx, 0, TOS - 1)], 0.0) / OSAMP
    M2 = M * s2[None, :]
    M2V = M2 * tsh[None, :]
    C_nn = (eta_nn.astype(f64) ** 2).sum(axis=(0, 2, 3))  # [3]
    sp_lam = np.logaddexp(0.0, float(lambda_reg.reshape(-1)[0]))
    creg = 2.0 * sp_lam / C_nn
    return M2, M2V, tsh, ctc_dc, C_dc, creg


# ---------------------------------------------------------------------------
# bass module
# ---------------------------------------------------------------------------

_NC_CACHE = {}


def _build_nc():
    if "nc" in _NC_CACHE:
        return _NC_CACHE["nc"]

    import concourse.mybir as mybir
    import concourse.tile as tile
    from concourse import bacc

    dt = mybir.dt.float32
    bf = mybir.dt.bfloat16
    Alu = mybir.AluOpType
    Act = mybir.ActivationFunctionType
    Ax = mybir.AxisListType

    nc = bacc.Bacc("TRN2", target_bir_lowering=False, debug=False)

    # shared constants
    d_argw = nc.declare_dram_parameter("argw", [2 * TILES, NQ * P], bf,
                                       isOutput=False)
    d_ident = nc.declare_dram_parameter("ident", [P, P], bf, isOutput=False)
    d_qqv3 = nc.declare_dram_parameter("qqv3", [P, 4 * 3 * NB], bf,
                                       isOutput=False)
    # per-core data
    d_w3h = nc.declare_dram_parameter("w3h", [P, TILES * 3 * NB], bf,
                                      isOutput=False)
    d_eta0 = nc.declare_dram_parameter("eta0", [P, 3 * TILES], dt,
                                       isOutput=False)
    d_cpl48 = nc.declare_dram_parameter("cpl48", [P, 3 * TILES], dt,
                                        isOutput=False)
    d_s48 = nc.declare_dram_parameter("s48", [P, 3 * TILES], dt, isOutput=False)
    d_out = nc.declare_dram_parameter("out", [P, 3 * TILES], dt, isOutput=True)

    with tile.TileContext(nc) as tc:
        with (
            tc.tile_pool(name="const", bufs=1) as cpool,
            tc.tile_pool(name="state", bufs=2) as spool,
            tc.tile_pool(name="iter", bufs=2) as ipool,
            tc.tile_pool(name="quad", bufs=4) as qpool,
            tc.tile_pool(name="small", bufs=2) as mpool,
            tc.tile_pool(name="ps_arg", bufs=2, space="PSUM") as ps_arg,
            tc.tile_pool(name="ps_y3", bufs=3, space="PSUM") as ps_y3,
            tc.tile_pool(name="ps_spx", bufs=2, space="PSUM") as ps_spx,
            tc.tile_pool(name="ps_kn", bufs=1, space="PSUM") as ps_kn,
        ):
            # ---- load constants ----
            argw = cpool.tile([2 * TILES, NQ * P], bf, tag="argw")
            nc.gpsimd.dma_start(argw[:], d_argw[:])
            ident = cpool.tile([P, P], bf, tag="ident")
            nc.gpsimd.dma_start(ident[:], d_ident[:])
            qqv3 = cpool.tile([P, 4 * 3 * NB], bf, tag="qqv3")
            nc.gpsimd.dma_start(qqv3[:], d_qqv3[:])
            w3h = cpool.tile([P, TILES * 3 * NB], bf, tag="w3h")
            nc.gpsimd.dma_start(w3h[:], d_w3h[:])
            cpl48 = cpool.tile([P, 3 * TILES], dt, tag="cpl48")
            nc.gpsimd.dma_start(cpl48[:], d_cpl48[:])
            s48 = cpool.tile([P, 3 * TILES], dt, tag="s48")
            nc.gpsimd.dma_start(s48[:], d_s48[:])
            eta_in = cpool.tile([P, 3 * TILES], dt, tag="eta_in")
            nc.gpsimd.dma_start(eta_in[:], d_eta0[:])

            eta48 = spool.tile([P, 3 * TILES], dt, tag="eta48")
            nc.vector.tensor_copy(eta48[:], eta_in[:])

            for it in range(MAX_ITER):
                eA = eta48[:, 0:TILES]
                eK = eta48[:, TILES:2 * TILES]
                eT = eta48[:, 2 * TILES:3 * TILES]

                # --- eta-only prefolds (off the serial tail) ---
                # kn[:, 2t]=(k*t0)_t, kn[:, 2t+1]=(-k)_t; knT = kn^T
                kn = ipool.tile([P, 2 * TILES], bf, tag="kn")
                nc.vector.tensor_tensor(kn[:, 0:2 * TILES:2], eK, eT, Alu.mult)
                nc.vector.tensor_scalar_mul(kn[:, 1:2 * TILES:2], eK, -1.0)
                knt_ps = ps_kn.tile([2 * TILES, P], bf, tag="kntp")
                nc.tensor.transpose(knt_ps[:], kn[:], ident[:])
                knT = ipool.tile([2 * TILES, P], bf, tag="knT")
                nc.scalar.copy(knT[:], knt_ps[:])
                # A3 = [A|A|A], AK = A*k  (GpSimd, parallel)
                A3 = ipool.tile([P, 3 * TILES], dt, tag="A3")
                for c in range(3):
                    nc.gpsimd.tensor_copy(A3[:, c * TILES:(c + 1) * TILES], eA)
                AK = ipool.tile([P, TILES], dt, tag="AK")
                nc.gpsimd.tensor_tensor(AK[:], eA, eK, Alu.mult)
                # upc = eta*s48 + cpl48 ; m48 = -2LR*min(eta,0)
                ups = ipool.tile([P, 3 * TILES], dt, tag="ups")
                nc.gpsimd.tensor_tensor(ups[:], eta48[:], s48[:], Alu.mult)
                upc = ipool.tile([P, 3 * TILES], dt, tag="upc")
                nc.gpsimd.tensor_tensor(upc[:], ups[:], cpl48[:], Alu.add)
                m48 = ipool.tile([P, 3 * TILES], dt, tag="m48")
                nc.gpsimd.tensor_scalar(m48[:], eta48[:], 0.0, -2.0 * LR,
                                        Alu.min, Alu.mult)

                # --- per-quad pipeline: arg -> sigma -> {y, sigma^T} ---
                ssd3 = ipool.tile([P, TILES * 3 * NB], bf, tag="ssd3")
                sdacc = ipool.tile([P, 2 * NQ], dt, tag="sdacc")
                ph1 = ipool.tile([P, TILES * 3 * NB], bf, tag="ph1")
                for q in range(NQ):
                    argp = ps_arg.tile([P, P], dt, tag="argp")
                    nc.tensor.matmul(
                        argp[:], argw[:, q * P:(q + 1) * P], knT[:],
                        start=True, stop=True,
                    )
                    s1Tq = qpool.tile([P, P], bf, tag="s1Tq")
                    nc.scalar.activation(s1Tq[:], argp[:], Act.Sigmoid)
                    y3p = ps_y3.tile([P, 4 * 3 * NB], dt, tag="y3p")
                    nc.tensor.matmul(
                        y3p[:], s1Tq[:], qqv3[:],
                        start=True, stop=True,
                    )
                    spxp = ps_spx.tile([P, P], bf, tag="spxp")
                    nc.tensor.transpose(spxp[:], s1Tq[:], ident[:])

                    qof = q * 4 * 3 * NB
                    s3q = ssd3[:, qof:qof + 4 * 3 * NB] \
                        .rearrange("p (a c b) -> p a c b", a=4, c=3)
                    # s1 (valid 16 of each 32-slot)
                    nc.scalar.copy(
                        s3q[:, :, 0, :],
                        spxp[:].rearrange("p (a b) -> p a b", a=4)[:, :, 0:NB],
                    )
                    # sd = s1*(1-s1) into slots 1 and 2 (two amr, same engine
                    # so no cross-engine hop; custom-DVE APs max 2 free dims)
                    nc.vector.affine_mul_reduce(
                        s3q[:, :, 1, :], sdacc[:, q:q + 1],
                        s3q[:, :, 0, :], s3q[:, :, 0, :], -1.0, 1.0,
                    )
                    nc.vector.affine_mul_reduce(
                        s3q[:, :, 2, :], sdacc[:, NQ + q:NQ + q + 1],
                        s3q[:, :, 0, :], s3q[:, :, 0, :], -1.0, 1.0,
                    )
                    # y-side products straight off PSUM (frees ScalarE)
                    nc.vector.tensor_tensor(
                        ph1[:, qof:qof + 4 * 3 * NB], y3p[:],
                        ssd3[:, qof:qof + 4 * 3 * NB], Alu.mult)

                # --- dots per half (8 tiles) ---
                # DD padded to 112 so the strided out-view slices stay
                # in-bounds; layout DD[:, d*16 + t], d in 0..5
                DD = mpool.tile([P, 7 * TILES], dt, tag="DD")
                for h in range(2):
                    hof = h * 8 * 3 * NB
                    hsl = slice(hof, hof + 8 * 3 * NB)
                    # w-side products on GpSimd (off the DVE)
                    ph2 = mpool.tile([P, 8 * 3 * NB], bf, tag=f"ph2_{h}")
                    nc.gpsimd.tensor_tensor(ph2[:], w3h[:, hsl], ssd3[:, hsl],
                                            Alu.mult)
                    # DD[:, d*16 + (8h+t)] <- sum_B ph[:, t, d, B]
                    nc.vector.tensor_reduce(
                        DD[:, 8 * h:8 * h + 48]
                        .rearrange("p (c t) -> p t c", c=3)[:, 0:8, :],
                        ph1[:, hsl].rearrange("p (t c b) -> p t c b", t=8, c=3),
                        Ax.X, Alu.add)
                    nc.vector.tensor_reduce(
                        DD[:, 3 * TILES + 8 * h:3 * TILES + 8 * h + 48]
                        .rearrange("p (c t) -> p t c", c=3)[:, 0:8, :],
                        ph2[:].rearrange("p (t c b) -> p t c b", t=8, c=3),
                        Ax.X, Alu.add)

                # --- combine on DVE (short chain) ---
                GUp = mpool.tile([P, 3 * TILES], dt, tag="GUp")
                nc.vector.tensor_tensor(GUp[:], A3[:], DD[:, 0:3 * TILES],
                                        Alu.mult)
                GU = mpool.tile([P, 3 * TILES], dt, tag="GU")
                nc.vector.tensor_tensor(GU[:], GUp[:],
                                        DD[:, 3 * TILES:6 * TILES],
                                        Alu.subtract)
                U_ap = GU[:, TILES:2 * TILES]
                V_ap = GU[:, 2 * TILES:3 * TILES]
                G48 = mpool.tile([P, 3 * TILES], dt, tag="G48")
                nc.vector.tensor_copy(G48[:, 0:TILES], GU[:, 0:TILES])
                nc.vector.tensor_tensor(G48[:, 2 * TILES:3 * TILES], AK[:],
                                        U_ap, Alu.mult)
                m1 = mpool.tile([P, TILES], dt, tag="m1")
                nc.vector.tensor_tensor(m1[:], eT, U_ap, Alu.mult)
                m2 = mpool.tile([P, TILES], dt, tag="m2")
                nc.vector.tensor_tensor(m2[:], m1[:], V_ap, Alu.subtract)
                nc.vector.tensor_tensor(G48[:, TILES:2 * TILES], eA, m2[:],
                                        Alu.mult)

                # --- update: eta' = upc + (G48*(-LR) + m48) ---
                t48 = mpool.tile([P, 3 * TILES], dt, tag="t48")
                nc.vector.affine_then_add(t48[:], G48[:], m48[:], -LR, 0.0)
                eta48n = spool.tile([P, 3 * TILES], dt, tag="eta48")
                nc.vector.tensor_tensor(eta48n[:, TILES:3 * TILES],
                                        upc[:, TILES:3 * TILES],
                                        t48[:, TILES:3 * TILES], Alu.add)
                nc.vector.tensor_tensor(eta48n[:, 0:TILES], upc[:, 0:TILES],
                                        t48[:, 0:TILES], Alu.add)
                eta48 = eta48n

            nc.gpsimd.dma_start(d_out[:], eta48[:])

    nc.finalize()
    _NC_CACHE["nc"] = nc
    return nc


# ---------------------------------------------------------------------------
# public entry point
# ---------------------------------------------------------------------------

def _make_in_maps(ctc, aif, time, eta_nn, lambda_reg):
    f32 = np.float32
    M2, M2V, tsh, ctc_dc, C_dc, creg = _preprocess(
        ctc, aif, time, eta_nn, lambda_reg)

    toc = 2.0 / C_dc
    M2b = M2.reshape(T, NB, TOS // NB).sum(-1)     # [64, 16]
    M2Vb = M2V.reshape(T, NB, TOS // NB).sum(-1)
    cB = tsh.reshape(NB, TOS // NB).mean(-1)       # [16]
    Qq = toc * (M2b.T @ M2b)                       # [16, 16]
    Qv = toc * (M2b.T @ M2Vb)

    import ml_dtypes
    bf16 = ml_dtypes.bfloat16

    # argw[r, q*128 + 32a + B]: tile t = 4q+a; r==2t -> 1, r==2t+1 -> cB[B]
    cBp = cB[np.minimum(np.arange(BLK), NB - 1)]   # padded to 32
    argw = np.zeros((2 * TILES, NQ * P), bf16)
    for q_ in range(NQ):
        for a_ in range(4):
            t_ = 4 * q_ + a_
            col = q_ * P + 32 * a_
            argw[2 * t_, col:col + 32] = 1.0
            argw[2 * t_ + 1, col:col + 32] = cBp.astype(bf16)
    ident = np.eye(P, dtype=bf16)
    # block-diagonal compact Gram stack:
    # qqv3[32a+B, 48a + 16d + j] = [Qq|Qq|Qv][B, 16d+j], B < 16
    blkrow = np.concatenate([Qq, Qq, Qv], axis=1)          # [16, 48]
    qqv3 = np.zeros((P, 4 * 3 * NB), bf16)                 # [128, 192]
    for a_ in range(4):
        qqv3[32 * a_:32 * a_ + NB, 48 * a_:48 * a_ + 48] = \
            blkrow.astype(bf16)

    s48 = np.zeros((P, 3 * TILES), f32)
    for c in range(3):
        s48[:, c * TILES:(c + 1) * TILES] = 1.0 - LR * creg[c]

    in_maps = []
    for m in range(N_CORES):
        rows = slice(m * ROWS_PER_CORE, (m + 1) * ROWS_PER_CORE)
        cd = ctc_dc[rows]                         # [16, 128, 64]
        wq = toc * (cd @ M2b)                     # [16, 128, 16]
        wv = toc * (cd @ M2Vb)
        wq_pm = wq.transpose(1, 0, 2)             # [128, 16, 16]
        wv_pm = wv.transpose(1, 0, 2)
        w3h = np.ascontiguousarray(
            np.stack([wq_pm, wq_pm, wv_pm], axis=2)
            .reshape(P, TILES * 3 * NB)).astype(bf16)
        pr = eta_nn[0, :, rows, :].astype(np.float64)   # [3, 16, 128]
        eta0 = np.ascontiguousarray(
            pr.transpose(2, 0, 1).reshape(P, 3 * TILES)).astype(f32)
        cpl48 = np.zeros((P, 3 * TILES), f32)
        for c in range(3):
            cpl48[:, c * TILES:(c + 1) * TILES] = (LR * creg[c] * pr[c]).T
        in_maps.append({
            "argw": argw, "ident": ident, "qqv3": qqv3, "w3h": w3h,
            "eta0": eta0, "cpl48": cpl48, "s48": s48,
        })
    return in_maps


def kernel(ctc, aif, time, seg, eta_nn, lambda_reg):
    from concourse.bass_utils import run_bass_kernel_spmd

    ctc = np.asarray(ctc)
    aif = np.asarray(aif)
    time = np.asarray(time)
    eta_nn = np.asarray(eta_nn)
    lambda_reg = np.asarray(lambda_reg)

    in_maps = _make_in_maps(ctc, aif, time, eta_nn, lambda_reg)
    nc = _build_nc()
    res = run_bass_kernel_spmd(nc, in_maps, list(range(N_CORES)))

    out = np.zeros((1, 3, H, W), np.float32)
    for m in range(N_CORES):
        rows = slice(m * ROWS_PER_CORE, (m + 1) * ROWS_PER_CORE)
        arr = res.results[m]["out"]                  # [128, 48]
        out[0, :, rows, :] = arr.reshape(P, 3, TILES).transpose(1, 2, 0)
    return out
